# revision 7
# baseline (speedup 1.0000x reference)
"""Causal single-head attention (B=4, T=4096, C=1024, D=64) on 8 NeuronCores.

Sharding: core c = (batch b = c % 4, half h = c // 4).
Each core handles ALL queries of its batch against its half of the key
blocks (256-token blocks with block index ≡ h mod 2).  Pure SPMD: the
program is identical on every core; cores differ only in input data
(x[b]^T, block-pair-permuted for h=1, and the causal mask).  Each core
emits unnormalized U[q, 0:64] = sum_k exp(s) v and U[q, 64] = sum_k
exp(s); the host adds the two halves per batch and normalizes.

On-chip dataflow (bf16 compute, f32 PSUM accumulation):
  Key tiles (even tt): ONE stacked matmul per c-tile with w stationary
    produces [Q^T; K^T] [128, 128t] directly (rows 0:64 = Q^T, 64:128 =
    K^T) -- no PE transposes; DVE copies qT / kT (partition-shifted)
    out of PSUM.  The V projection ([128t, 64], x stationary) is
    DEFERRED into the pair loop of its own supertile: the prefix of the
    run is PE-bound (projections crowd out S matmuls that feed the
    ACT), while the suffix is ACT-bound with PE slack.
  Q-only tiles (odd tt): supertile 0 uses the direct path (w
    stationary, [64, 128t] out); later supertiles use the cheaper fused
    path ([128t, 64] out) with the two odd tiles of a supertile batched
    through ONE [128,128] PE transpose (qT copies partition-shifted).
  per query supertile st (512 q), local key tile pairs jp (diag first):
    S^T [128k, 2x512q] = K^T[j] @ Q^T[st]     (contraction over D=64)
    P^T = exp(S^T / 8)                        (one ACT instr per pair)
    diagonal pair: P^T *= mask                (DVE)
    U [128q, 65] += P^T-chunk^T @ [V_j | 1]   (P chunk stationary,
        deferred TWO pairs so the PE stays off the exp critical path;
        the deferral queue is drained early during st=7, and the very
        last exp is split in half, to cut the serial tail)
  U -> SBUF -> DMA out per supertile (SWDGE/Pool path for st<7,
  keeping HWDGE free for input streaming).
"""
import sys
import numpy as np
import ml_dtypes

if "/opt/trn_rl_repo" not in sys.path:
    sys.path.insert(0, "/opt/trn_rl_repo")

import concourse.bacc as bacc
import concourse.mybir as mybir
from concourse import tile
from concourse import bass_utils

bf16 = mybir.dt.bfloat16
f32 = mybir.dt.float32
BF = ml_dtypes.bfloat16

B, T, C, D = 4, 4096, 1024, 64
NC_ = C // 128      # 8 c-tiles
NTT = T // 128      # 32 token tiles
NST = 8             # query supertiles (512 q each)
STQ = 512

_CACHE = {}


def _build():
    nc = bacc.Bacc(None, target_bir_lowering=False, debug=False, num_devices=8)

    # xq tile-packed: xq[:, 1024*tt + 128*c : +128] = x^T[c-tile, token tile tt]
    xq = nc.dram_tensor("xq", [128, NTT * 1024], bf16, kind="ExternalInput")
    # w repacked: [wqk (8 c-tiles x 128) | wv (8 x 64)] so the critical
    # qk part can be DMA'd before the first x tiles and wv after them
    w = nc.dram_tensor("w", [128, NC_ * 192], bf16, kind="ExternalInput")
    # aux2 = diag mask [0:768] | identity [768:896]
    aux2 = nc.dram_tensor("aux2", [128, 896], bf16, kind="ExternalInput")
    out = nc.dram_tensor("out", [128, NST * 260], bf16, kind="ExternalOutput")

    with tile.TileContext(nc) as tc:
        with tc.tile_pool(name="sb", bufs=1) as sb, \
             tc.tile_pool(name="qk", bufs=3) as qkp, \
             tc.tile_pool(name="pp", bufs=7) as pp, \
             tc.tile_pool(name="uo", bufs=2) as uop, \
             tc.tile_pool(name="ps", bufs=2, space="PSUM") as ps:

            # ---- resident tiles ----
            xq_sb = sb.tile([128, NTT * 1024], bf16, tag="xq")
            w_sb = sb.tile([128, NC_ * 192], bf16, tag="w")
            aux_sb = sb.tile([128, 896], bf16, tag="aux")
            msk_sb = aux_sb[:, 0:768]
            idn_sb = aux_sb[:, 768:896]
            qT = sb.tile([64, T], bf16, tag="qT")       # Q^T strip
            kT = sb.tile([64, T // 2], bf16, tag="kT")  # K^T local tiles
            v_sb = sb.tile([128, 16 * 65], bf16, tag="v")  # [V_j | 1] tiles

            def wqk(c):
                return w_sb[:, 128 * c:128 * (c + 1)]

            def wv(c):
                return w_sb[:, 1024 + 64 * c:1024 + 64 * (c + 1)]

            # DMA order = consumption order: wqk, the first supertile's x
            # tiles, then aux (masks/idn), wv, and the remaining x tiles.
            nc.sync.dma_start(w_sb[:, 0:1024], w[:, 0:1024])
            nc.sync.dma_start(w_sb[:, 1024:1536], w[:, 1024:1536])
            for tt in range(4):
                nc.sync.dma_start(xq_sb[:, 1024 * tt:1024 * (tt + 1)],
                                  xq[:, 1024 * tt:1024 * (tt + 1)])
            nc.sync.dma_start(aux_sb[:], aux2[:])
            for tt in range(4, NTT):
                nc.sync.dma_start(xq_sb[:, 1024 * tt:1024 * (tt + 1)],
                                  xq[:, 1024 * tt:1024 * (tt + 1)])

            # warm/trig memsets on Pool so the DVE queue stays empty and the
            # PE warmup can launch at ~0.8us
            warm = sb.tile([128, 640], bf16, tag="warm")
            trig = sb.tile([128, 1], bf16, tag="trig")
            nc.gpsimd.memset(warm[:], 0.0)
            nc.gpsimd.memset(trig[:], 0.0)
            # ones columns of the V tiles (denominator trick)
            for j in range(16):
                nc.gpsimd.memset(v_sb[:, 65 * j + 64:65 * j + 65], 1.0)

            # trigger the ACT Exp table load (1.28us) at t=0, not at the
            # first real exp
            nc.scalar.activation(trig[:], trig[:],
                                 mybir.ActivationFunctionType.Exp, scale=1.0)
            # PE p-state warmup: the PE ramps to full clock only after ~3us
            # of continuous busy; bridge the DMA wait with junk matmuls.
            for i in range(5):
                wps = ps.tile([128, 512], f32, tag="s", name=f"warm{i}")
                nc.tensor.matmul(wps[:], warm[:, 0:128], warm[:, 128:640],
                                 start=True, stop=True)

            def xsrc(tt, c):
                return xq_sb[:, 1024 * tt + 128 * c:1024 * tt + 128 * (c + 1)]

            # ---- projection units ----
            def key_unit(tt):
                """Stacked [Q^T; K^T] direct matmul for an even tile."""
                j = tt // 2
                pj = ps.tile([128, 128], f32, tag="pj", name=f"pjk{tt}",
                             bufs=3)
                for c in range(NC_):
                    # w stationary: out rows 0:64 = Q^T, 64:128 = K^T
                    nc.tensor.matmul(pj[:], wqk(c), xsrc(tt, c),
                                     start=(c == 0), stop=(c == NC_ - 1))
                nc.vector.tensor_copy(kT[:, 128 * j:128 * (j + 1)],
                                      pj[64:128, :])
                nc.vector.tensor_copy(qT[:, 128 * tt:128 * (tt + 1)],
                                      pj[0:64, :])
                v_unit(tt)

            def v_unit(tt):
                """Deferred V projection ([token, d] layout, x stationary)."""
                j = tt // 2
                pjv = ps.tile([128, 64], f32, tag="pj", name=f"pjv{tt}",
                              bufs=3)
                for c in range(NC_):
                    nc.tensor.matmul(pjv[:], xsrc(tt, c), wv(c),
                                     start=(c == 0), stop=(c == NC_ - 1))
                nc.vector.tensor_copy(v_sb[:, 65 * j:65 * j + 64], pjv[:])

            def q_direct(tt):
                """Direct Q^T (w stationary) -- shortest latency chain."""
                pjq = ps.tile([64, 128], f32, tag="pj", name=f"pjq{tt}",
                              bufs=3)
                for c in range(NC_):
                    nc.tensor.matmul(pjq[:], wqk(c)[:, 0:64], xsrc(tt, c),
                                     start=(c == 0), stop=(c == NC_ - 1))
                nc.vector.tensor_copy(qT[:, 128 * tt:128 * (tt + 1)], pjq[:])

            def q_pair_A(st):
                """Fused Q projection of tile 4st+1 into a shared pj bank."""
                t1 = 4 * st + 1
                pj = ps.tile([128, 128], f32, tag="pj", name=f"pjp{st}",
                             bufs=3)
                for c in range(NC_):
                    nc.tensor.matmul(pj[:, 0:64], xsrc(t1, c),
                                     wqk(c)[:, 0:64],
                                     start=(c == 0), stop=(c == NC_ - 1),
                                     skip_group_check=True)
                return pj

            def q_pair_B(st, pj):
                """Tile 4st+3 projection + ONE batched transpose + copies."""
                t1, t2 = 4 * st + 1, 4 * st + 3
                for c in range(NC_):
                    nc.tensor.matmul(pj[:, 64:128], xsrc(t2, c),
                                     wqk(c)[:, 0:64],
                                     start=False, stop=(c == NC_ - 1),
                                     skip_group_check=True)
                qk2 = qkp.tile([128, 128], bf16, tag="qk", name=f"qk{st}")
                nc.vector.tensor_copy(qk2[:], pj[:])
                tp = ps.tile([128, 128], bf16, tag="pj", name=f"tp{st}",
                             bufs=3)
                nc.tensor.transpose(tp[:], qk2[:], idn_sb)
                nc.vector.tensor_copy(qT[:, 128 * t1:128 * (t1 + 1)],
                                      tp[0:64, :])
                nc.vector.tensor_copy(qT[:, 128 * t2:128 * (t2 + 1)],
                                      tp[64:128, :])

            # ---- attention: one continuous pair stream across supertiles ----
            # Global software pipeline: the U matmuls of a pair are deferred
            # two pairs (possibly crossing into the next supertile) so the
            # PE never sits on the ACT exp critical path, and the ACT stream
            # has no supertile-boundary bubble.
            u_state = {}   # st -> (u4 tile, n_emitted)
            pendings = []  # (st, jp, p2, ds)

            def emit_u_d(st, jp, p2, d):
                if st not in u_state:
                    u_state[st] = [ps.tile([128, 260], f32, tag="u",
                                           name=f"u{st}", bufs=1), 0]
                ent = u_state[st]
                u4 = ent[0]
                j = 2 * jp + d
                total = 8 * (st + 1) - 2  # diag d1 contributes only g=2,3
                if jp == st and d == 1:
                    gs = [(2, 512), (3, 640)]
                else:
                    gs = [(g, 512 * d + 128 * g) for g in range(4)]
                for g, lo in gs:
                    # start=True zeroes the WHOLE PSUM bank: set it only
                    # on the chronologically first matmul into u4.
                    nc.tensor.matmul(
                        u4[:, 65 * g:65 * (g + 1)],
                        p2[:, lo:lo + 128],
                        v_sb[:, 65 * j:65 * (j + 1)],
                        start=(ent[1] == 0),
                        stop=(ent[1] == total - 1),
                        skip_group_check=True)
                    ent[1] += 1

            def ship(st):
                ent = u_state[st]
                if ent[1] == 8 * (st + 1) - 2:  # supertile complete -> ship
                    uo_t = uop.tile([128, 260], bf16, tag="uo", name=f"uo{st}")
                    if st <= 3 or st == NST - 1:
                        nc.scalar.activation(uo_t[:], ent[0][:],
                                             mybir.ActivationFunctionType.Copy)
                    else:
                        nc.vector.tensor_copy(uo_t[:], ent[0][:])
                    eng = nc.sync if st == NST - 1 else nc.gpsimd
                    eng.dma_start(out[:, 260 * st:260 * (st + 1)], uo_t[:])
                    del u_state[st]

            def emit_u(st, jp, p2, ds=(0, 1)):
                if jp == st and 0 in ds:  # diagonal pair -> causal mask,
                    # deferred here so the DVE is free at the boundary
                    nc.vector.tensor_mul(p2[:, 0:768], p2[:, 0:768], msk_sb)
                for d in ds:
                    emit_u_d(st, jp, p2, d)
                ship(st)

            def emit_pair(st, jp, cap=5, split_exp=False):
                qsl = slice(STQ * st, STQ * (st + 1))
                s2 = ps.tile([128, 1024], f32, tag="s", name=f"s{st}_{jp}")
                p2 = pp.tile([128, 1024], bf16, tag="p", name=f"p{st}_{jp}")
                if jp == st:
                    # diagonal pair: with 128-interleaved keys, the second
                    # tile is visible only to queries [256:512) for BOTH
                    # halves -> 768 live columns instead of 1024
                    nc.tensor.matmul(s2[:, 0:512],
                                     kT[:, 128 * 2 * jp:128 * (2 * jp + 1)],
                                     qT[:, qsl], start=True, stop=True)
                    nc.tensor.matmul(s2[:, 512:768],
                                     kT[:, 128 * (2 * jp + 1):128 * (2 * jp + 2)],
                                     qT[:, STQ * st + 256:STQ * (st + 1)],
                                     start=True, stop=True)
                    nc.scalar.activation(p2[:, 0:768], s2[:, 0:768],
                                         mybir.ActivationFunctionType.Exp,
                                         scale=0.125)
                    pendings.append((st, jp, p2, (0, 1)))
                else:
                    for d in range(2):
                        j = 2 * jp + d
                        nc.tensor.matmul(s2[:, 512 * d:512 * (d + 1)],
                                         kT[:, 128 * j:128 * (j + 1)],
                                         qT[:, qsl], start=True, stop=True)
                    if split_exp:
                        # last pair of the run: exp in halves so the final
                        # U matmuls overlap the second half's exp
                        for d in range(2):
                            nc.scalar.activation(
                                p2[:, 512 * d:512 * (d + 1)],
                                s2[:, 512 * d:512 * (d + 1)],
                                mybir.ActivationFunctionType.Exp, scale=0.125)
                            pendings.append((st, jp, p2, (d,)))
                    else:
                        nc.scalar.activation(p2[:], s2[:],
                                             mybir.ActivationFunctionType.Exp,
                                             scale=0.125)
                        pendings.append((st, jp, p2, (0, 1)))
                while len(pendings) > cap:
                    emit_u(*pendings.pop(0))

            # ---- schedule ----
            key_unit(0)
            q_direct(1)
            key_unit(2)
            q_direct(3)

            qp_live = {}  # st -> pj tile from q_pair_A

            def fillers_for(st):
                """Fillers to interleave into pairs of supertile st: the
                projection units of supertile st+1 (in tile-arrival order)
                then the deferred V units of supertile st itself."""
                fl = []
                if st + 1 < NST:
                    n = st + 1
                    fl += [lambda: key_unit(4 * n),
                           lambda: qp_live.__setitem__(n, q_pair_A(n)),
                           lambda: key_unit(4 * n + 2),
                           lambda: q_pair_B(n, qp_live.pop(n))]
                return fl

            for st in range(NST):
                fillers = fillers_for(st)
                for pi, jp in enumerate([st] + list(range(st))):  # diag first
                    # during the last supertile, drain the deferral queue
                    # early so the tail after the final exp is short
                    cap = 5 if st < NST - 1 else max(1, 5 - pi)
                    emit_pair(st, jp, cap=cap,
                              split_exp=(st == NST - 1 and pi == st))
                    if fillers:
                        fillers.pop(0)()
                while fillers:
                    fillers.pop(0)()
            for pd in pendings:
                emit_u(*pd)
            pendings.clear()

    nc.compile()
    return nc


def _get_nc():
    if "nc" not in _CACHE:
        _CACHE["nc"] = _build()
    return _CACHE["nc"]


def kernel(x, Wq, Wk, Wv, _trace=False):
    x = np.asarray(x)
    nc = _get_nc()

    # Token permutation per half: the program treats EVEN 128-token tiles
    # as key tiles.  For h=1 cores we swap each adjacent tile pair so THEIR
    # key tiles land on even positions.
    tok = np.arange(T)
    perm1 = 128 * ((tok // 128) ^ 1) + tok % 128  # swap adjacent 128-tiles

    xT = np.ascontiguousarray(x.transpose(0, 2, 1)).astype(BF)   # [B, C, T]
    xT1 = np.ascontiguousarray(xT[:, :, perm1])

    w_all = np.concatenate([Wq, Wk, Wv], axis=1).astype(np.float32)  # [C, 192]
    w_blocks = w_all.reshape(NC_, 128, 192).transpose(1, 0, 2)  # [128, 8, 192]
    w_packed = np.ascontiguousarray(np.concatenate(
        [w_blocks[:, :, 0:128].reshape(128, NC_ * 128),
         w_blocks[:, :, 128:192].reshape(128, NC_ * 64)], axis=1)).astype(BF)
    idn = np.eye(128, dtype=BF)

    # Masks for the diagonal pair: program key tile A holds global tile
    # 4st+h, tile B holds 4st+2+h; program query quarter g holds global
    # tile 4st+(g^h).  Causal test on global ids:
    #   A: 128h + k <= 128(g^h) + i   (cols 0:512, all four quarters)
    #   B: 128(2+h) + k <= 128(g^h) + i   (cols 512:768, quarters g=2,3)
    i = np.arange(128)[None, :]
    k = np.arange(128)[:, None]
    masks = {}
    for h in range(2):
        colsA = [(128 * h + k <= 128 * (g ^ h) + i) for g in range(4)]
        colsB = [(128 * (2 + h) + k <= 128 * (g ^ h) + i) for g in (2, 3)]
        masks[h] = np.concatenate(colsA + colsB, axis=1).astype(BF)  # [128,768]

    def pack_tiles(xTb):
        # [C, T] -> [128, tt*1024 + c*128 + t]
        return np.ascontiguousarray(
            xTb.reshape(NC_, 128, NTT, 128).transpose(1, 2, 0, 3)
            .reshape(128, NTT * 1024))

    in_maps = []
    for c in range(8):
        b, h = c % 4, c // 4
        xTb = xT[b] if h == 0 else xT1[b]
        in_maps.append({
            "xq": pack_tiles(xTb),
            "w": w_packed,
            "aux2": np.concatenate([masks[h], idn], axis=1),
        })

    res = bass_utils.run_bass_kernel_spmd(nc, in_maps, core_ids=list(range(8)),
                                          trace=_trace)
    _CACHE["last_results"] = res

    # Decode: U[c] [128, 8*260] -> [q_perm, 65]; un-permute h=1 tokens.
    O = np.empty((B, T, D), dtype=np.float32)
    for b in range(B):
        Uh = []
        for h in range(2):
            U = np.asarray(res.results[b + 4 * h]["out"],
                           dtype=np.float32)            # [128, 2080]
            U = U.reshape(128, NST, 4, 65).transpose(1, 2, 0, 3)
            U = U.reshape(T, 65)                          # permuted q order
            Uh.append(U[perm1] if h == 1 else U)          # global q order
        Ut = Uh[0] + Uh[1]
        O[b] = Ut[:, 0:64] / Ut[:, 64:65]
    return O
